# revision 41
# baseline (speedup 1.0000x reference)
"""Multi-head causal attention on 8 Trainium2 NeuronCores — single launch.

Problem (full shapes): x [4, 2048, 1024], wq/wk/wv [16, 1024, 64],
w_proj [1024, 1024], b_proj [1024] -> out [4, 2048, 1024].

Strategy (ONE SPMD launch; the dominant cost in this environment is the
fixed ~70ms relay overhead per launch, so attention + projection are
fused into a single NEFF with an on-device AllToAll between them):

Stage 1 — head-parallel attention. Each core owns 2 of the 16 heads.
Per core: QKV projections contract over C on the partition dim using a
host-pretransposed xT; scores are computed transposed
(scoresT[s, tq] = kT_slice.T @ qT) so the exp'd weights land directly in
the [s, tq] layout the PE needs as the stationary operand of wei @ v;
causal block skipping on both the scores and the wei@v matmuls. The
softmax denominator comes free from a ones-column appended to V (the
wei@v matmul computes [v | 1].T @ wei, row 64 = sum of weights). Exp on
the scalar engine reading PSUM directly, diagonal-block masking and
normalization on the vector engine. Matmul operands are bf16 (host-
rounded), accumulation fp32 in PSUM.

Stage 2 — pipelined AllToAlls. Per (batch, token-half), the half's
attention output is scattered into a DRAM bounce buffer group-by-group
as it is normalized, and a 0.25MB AllToAll redistributes it
channel-sharded -> token-sharded while later attention still computes.

Stage 3 — token-parallel output projection, interleaved: each 128-token
slab's projection (full w_proj, contraction over all 1024 channels) is
emitted where its AllToAll is guaranteed complete, keeping the PE busy;
only the final slab's collective is latency-exposed. fp32 output,
assembled (transpose + bias) on host.

kernel() is self-contained: hardcodes shapes, shards on host, runs the
single SPMD NEFF on cores 0-7, reassembles the full output on host.
"""

import numpy as np
import ml_dtypes

import concourse.bass as bass
import concourse.mybir as mybir
import concourse.tile as tile
from concourse.bass_utils import run_bass_kernel_spmd

B, T, C, H, D = 4, 2048, 1024, 16, 64
NCORES = 8
HPC = H // NCORES          # heads per core = 2
BT = B * T                 # 8192
TL = BT // NCORES          # 1024 tokens per core in stage 3
CB = C // 128              # 8 contraction blocks
NB = T // 128              # 16 s-blocks per batch
F32 = mybir.dt.float32
BF16 = mybir.dt.bfloat16
EXP = mybir.ActivationFunctionType.Exp
BF = ml_dtypes.bfloat16

_CACHE: dict = {}

# tuning knobs (read at build time)
TUNE = {
    "xt_bufs": 18,
    "wei_bufs": 4,
    "sc_bufs": 2,
    "av_bufs": 2,
    "qkv_ps_bufs": 2,
    "outT_bufs": 2,
}


def split_waits(nc, budget=1):
    """Walrus codegen rejects instructions carrying too many semaphore
    waits; offload excess waits onto preceding same-engine NOPs."""
    k = 0
    for bb in nc.main_func.blocks:
        insts = bb.instructions
        i = 0
        while i < len(insts):
            ins = insts[i]
            si = getattr(ins, "sync_info", None)
            if si is not None and si.on_wait and len(si.on_wait) > budget:
                waits = list(si.on_wait)
                extra, keep = waits[:-budget], waits[-budget:]
                pos = i
                for c in range(0, len(extra), budget):
                    nop = mybir.InstNoOp(
                        name=f"I-waitsplit{k}",
                        engine=ins.engine,
                        ins=[],
                        outs=[],
                        sync_info=mybir.SyncInfo(
                            on_wait=extra[c : c + budget], on_update=[]
                        ),
                        bass_nofuse=True,
                    )
                    k += 1
                    insts.insert(pos, nop)
                    pos += 1
                    i += 1
                ins.sync_info = mybir.SyncInfo(
                    on_wait=keep, on_update=list(si.on_update or [])
                )
            i += 1
    return k


def _build_fused(split=True, reps=1):
    # reps>1 unrolls the whole body (collectives cannot sit inside For_i
    # control flow) — used only by the --hwtime marginal-cost measurement.
    nc = bass.Bass()

    xT = nc.dram_tensor("xT", [C, BT], BF16, kind="ExternalInput")
    wq2 = nc.dram_tensor("wq2", [C, 128], BF16, kind="ExternalInput")
    wk2 = nc.dram_tensor("wk2", [C, 128], BF16, kind="ExternalInput")
    wv2 = nc.dram_tensor("wv2", [C, 128], BF16, kind="ExternalInput")
    wp = nc.dram_tensor("wp", [C, C], BF16, kind="ExternalInput")
    yT = nc.dram_tensor("yT", [C, TL], F32, kind="ExternalOutput")

    ident_d = nc.inline_tensor(np.eye(128, dtype=BF), name="ident")
    # mask[s, tq] = 1 where s <= tq (keep); applied to the diagonal block
    mask_d = nc.inline_tensor(
        np.triu(np.ones((128, 128), dtype=BF)), name="mask"
    )
    ones_d = nc.inline_tensor(np.ones((1, 64), dtype=BF), name="ones64")

    with tile.TileContext(nc) as tc:
        with (
            tc.tile_pool(name="wpool", bufs=1) as wpool,
            tc.tile_pool(name="qkv", bufs=2) as qkv_pool,
            tc.tile_pool(name="xp", bufs=6) as xpool,
            tc.tile_pool(name="wei", bufs=TUNE["wei_bufs"]) as wei_pool,
            tc.tile_pool(name="small", bufs=4) as spool,
            tc.tile_pool(name="outp", bufs=TUNE["outT_bufs"]) as opool,
            tc.tile_pool(name="ps_qkv", bufs=TUNE["qkv_ps_bufs"], space="PSUM") as ps_qkv,
            tc.tile_pool(name="ps_sc", bufs=TUNE["sc_bufs"], space="PSUM") as ps_sc,
            tc.tile_pool(name="ps_av", bufs=TUNE["av_bufs"], space="PSUM") as ps_av,
            tc.tile_pool(name="dram", bufs=1, space="DRAM") as dpool,
        ):
            # weights go through the Pool-engine DMA queue so the sync-engine
            # queue starts streaming x tiles immediately
            wq_sb = wpool.tile([128, CB, 128], BF16)
            wk_sb = wpool.tile([128, CB, 128], BF16)
            wv_sb = wpool.tile([128, CB, 128], BF16)
            nc.gpsimd.dma_start(wq_sb[:], wq2[:].rearrange("(b p) m -> p b m", p=128))
            nc.gpsimd.dma_start(wk_sb[:], wk2[:].rearrange("(b p) m -> p b m", p=128))
            nc.gpsimd.dma_start(wv_sb[:], wv2[:].rearrange("(b p) m -> p b m", p=128))
            ident = wpool.tile([128, 128], BF16)
            nc.gpsimd.dma_start(ident[:], ident_d[:])
            mask = wpool.tile([128, 128], BF16)
            nc.gpsimd.dma_start(mask[:], mask_d[:])
            ones64 = wpool.tile([1, 64], BF16)
            nc.gpsimd.dma_start(ones64[:], ones_d[:])
            # full projection weight, contraction-blocked (overlaps stage 1)
            wp_sb = wpool.tile([128, CB, C], BF16)
            nc.gpsimd.dma_start(wp_sb[:], wp[:].rearrange("(b p) o -> p b o", p=128))

            # Per-(batch, token-half) AllToAll bounce buffers (internal DRAM;
            # collectives cannot target kernel I/O tensors). For (b, jh),
            # chunk j of a2a_in[b][jh] holds this core's 128 channels for
            # tokens [2048b + 1024jh + 128j, ... + 128); after the AllToAll,
            # a2a_out[b][jh] holds all 1024 channels (chunk i = heads 2i,2i+1)
            # for this core's 128-token slab. Halving the collectives lets
            # each one launch mid-attention, hiding its latency.
            a2a_in = [
                [
                    dpool.tile([NCORES, 128, 128], BF16, name=f"a2ain{b}_{jh}")
                    for jh in range(2)
                ]
                for b in range(B)
            ]
            a2a_out = [
                [
                    dpool.tile([NCORES, 128, 128], BF16, name=f"a2aout{b}_{jh}")
                    for jh in range(2)
                ]
                for b in range(B)
            ]

            def emit_proj(b, jh):
                # projection for my 128-token slab (b, jh)
                att_sb = wpool.tile(
                    [128, CB, 128], BF16, tag="att", bufs=4, name=f"att{b}_{jh}"
                )
                nc.sync.dma_start(
                    att_sb[:], a2a_out[b][jh][:].rearrange("c p t -> p c t")
                )
                col0 = 256 * b + 128 * jh
                for mb in range(CB):
                    ps = ps_qkv.tile([128, 128], F32, tag="ps_qkv")
                    for cb in range(CB):
                        nc.tensor.matmul(
                            ps[:],
                            wp_sb[:, cb, 128 * mb : 128 * (mb + 1)],
                            att_sb[:, cb],
                            start=(cb == 0),
                            stop=(cb == CB - 1),
                        )
                    ot = opool.tile([128, 128], F32, tag="ot")
                    nc.any.tensor_copy(ot[:], ps[:])
                    nc.sync.dma_start(
                        yT[128 * mb : 128 * (mb + 1), col0 : col0 + 128],
                        ot[:],
                    )

            for _rep in range(reps):
              for b in range(B):
                t0 = b * T
                # ---- QKV for batch b, split into per-token-half tiles so
                # the second half's QKV has no (false, whole-tile) dependency
                # on the first half's attention reads ----
                qTh = [
                    qkv_pool.tile(
                        [128, T // 2], BF16, tag=f"qT{x}", name=f"qTh{x}"
                    )
                    for x in range(2)
                ]
                kTh = [
                    qkv_pool.tile(
                        [128, T // 2], BF16, tag=f"kT{x}", name=f"kTh{x}"
                    )
                    for x in range(2)
                ]
                v2h = [
                    qkv_pool.tile(
                        [128, NB // 2, 130], BF16, tag=f"v2{x}", name=f"v2h{x}"
                    )
                    for x in range(2)
                ]
                for x in range(2):
                    # ones columns at 64 (head 0) and 129 (head 1)
                    nc.gpsimd.memset(v2h[x][:, :, 64:65], 1.0)
                    nc.gpsimd.memset(v2h[x][:, :, 129:130], 1.0)

                def emit_qkv_tch(tch, t0=t0, qTh=qTh, kTh=kTh, v2h=v2h):
                    half, lc = tch // 2, tch % 2
                    tc0 = t0 + 512 * tch
                    xts = []
                    for cb in range(CB):
                        xt = xpool.tile([128, 512], BF16, tag="xt", bufs=TUNE["xt_bufs"])
                        nc.sync.dma_start(
                            xt[:], xT[128 * cb : 128 * (cb + 1), tc0 : tc0 + 512]
                        )
                        xts.append(xt)
                    for w_sb, dst in ((wq_sb, qTh[half]), (wk_sb, kTh[half])):
                        ps = ps_qkv.tile([128, 512], F32, tag="ps_qkv")
                        for cb in range(CB):
                            nc.tensor.matmul(
                                ps[:],
                                w_sb[:, cb],
                                xts[cb][:],
                                start=(cb == 0),
                                stop=(cb == CB - 1),
                            )
                        nc.any.tensor_copy(
                            dst[:, 512 * lc : 512 * (lc + 1)], ps[:]
                        )
                    # v (both heads packed): vT2 then PE-transpose to [t, d]
                    ps = ps_qkv.tile([128, 512], F32, tag="ps_qkv")
                    for cb in range(CB):
                        nc.tensor.matmul(
                            ps[:],
                            wv_sb[:, cb],
                            xts[cb][:],
                            start=(cb == 0),
                            stop=(cb == CB - 1),
                        )
                    vt = xpool.tile([128, 512], BF16, tag="vt", bufs=3)
                    nc.any.tensor_copy(vt[:], ps[:])
                    pst = ps_qkv.tile([128, 512], BF16, tag="ps_qkv")
                    for tb in range(4):
                        nc.tensor.transpose(
                            pst[:, 128 * tb : 128 * (tb + 1)],
                            vt[:, 128 * tb : 128 * (tb + 1)],
                            ident[:],
                        )
                    pst3 = pst[:].rearrange("p (t d) -> p t d", d=128)
                    sb0 = 4 * lc
                    nc.vector.tensor_copy(
                        v2h[half][:, sb0 : sb0 + 4, 0:64], pst3[:, :, 0:64]
                    )
                    nc.vector.tensor_copy(
                        v2h[half][:, sb0 : sb0 + 4, 65:129], pst3[:, :, 64:128]
                    )

                # ---- attention for batch b ----
                outT = opool.tile([128, T], BF16, tag="outT")

                def emit_attn_half(
                    jh, b=b, qTh=qTh, kTh=kTh, v2h=v2h, outT=outT
                ):
                    for h in range(HPC):
                        hp = 64 * h
                        av = {}
                        for jj in range(2):
                            j = 2 * jh + jj
                            av[j] = ps_av.tile(
                                [128, 512], F32, tag="av", name=f"av{j}"
                            )

                        # scores are software-pipelined one s-block ahead of
                        # the wei@v consumer so the scalar engine's exp has a
                        # full block of PE work to hide behind
                        wts = {}

                        def emit_scores(i, jh=jh, hp=hp, wts=wts):
                            ts_ = max(1024 * jh, 128 * i)
                            w = 1024 * jh + 1024 - ts_
                            ps = ps_sc.tile([128, 1024], F32, tag="sc")
                            off = 0
                            kx = kTh[i // 8]
                            klc = 128 * (i % 8)
                            qx = qTh[jh]
                            qb = 1024 * jh
                            while off < w:
                                n = min(512, w - off)
                                nc.tensor.matmul(
                                    ps[:, off : off + n],
                                    kx[hp : hp + 64, klc : klc + 128],
                                    qx[
                                        hp : hp + 64,
                                        ts_ + off - qb : ts_ + off - qb + n,
                                    ],
                                    start=True,
                                    stop=True,
                                )
                                off += n
                            wt = wei_pool.tile([128, 1024], BF16, tag="wei")
                            nc.scalar.activation(wt[:, :w], ps[:, :w], EXP)
                            if 128 * i >= 1024 * jh:
                                # first 128 cols are the diagonal block
                                nc.vector.tensor_mul(
                                    wt[:, 0:128], wt[:, 0:128], mask[:]
                                )
                            wts[i] = (wt, ts_)

                        emit_scores(0)
                        for i in range(8 * jh + 8):
                            if i + 1 < 8 * jh + 8:
                                emit_scores(i + 1)
                            wt, ts_ = wts.pop(i)
                            for jj in range(2):
                                j = 2 * jh + jj
                                glo = max(512 * j, ts_)
                                ghi = 512 * j + 512
                                n = ghi - glo
                                if n <= 0:
                                    continue
                                i_last = min(4 * j + 3, 8 * jh + 7)
                                nc.tensor.matmul(
                                    av[j][0:65, glo - 512 * j : glo - 512 * j + n],
                                    v2h[i // 8][:, i % 8, 65 * h : 65 * h + 65],
                                    wt[:, glo - ts_ : glo - ts_ + n],
                                    start=(i == 0),
                                    stop=(i == i_last),
                                )
                                if i == i_last:
                                    # reciprocal of the denominator row, then
                                    # broadcast it across partitions 64..127
                                    # of the same PSUM bank via a K=1 ones
                                    # matmul, and normalize.
                                    r = spool.tile([1, 512], BF16, tag="recip")
                                    with nc.allow_low_precision(
                                        reason="softmax recip in bf16"
                                    ):
                                        nc.vector.reciprocal(
                                            r[:], av[j][64:65, :]
                                        )
                                    nc.tensor.matmul(
                                        av[j][64:128, :],
                                        ones64[:],
                                        r[:],
                                        start=True,
                                        stop=True,
                                    )
                                    ot_sl = outT[
                                        hp : hp + 64, 512 * j : 512 * (j + 1)
                                    ]
                                    # stage numerator to SBUF first: engines
                                    # may read at most ONE operand from PSUM
                                    # (NCC_IBVF027)
                                    nc.any.tensor_copy(ot_sl, av[j][0:64, :])
                                    nc.vector.tensor_mul(
                                        ot_sl, ot_sl, av[j][64:128, :]
                                    )
                                    if h == HPC - 1:
                                        # both heads of this 512-token group
                                        # are normalized: scatter its 4 a2a
                                        # chunks now, while attention still
                                        # runs
                                        jl4 = 4 * (j - 2 * jh)
                                        nc.sync.dma_start(
                                            a2a_in[b][jh][
                                                jl4 : jl4 + 4
                                            ].rearrange("j p c -> p j c"),
                                            outT[
                                                :, 512 * j : 512 * (j + 1)
                                            ].rearrange(
                                                "p (j c) -> p j c", c=128
                                            ),
                                        )

                        # last batch only: slip the previous batch's second-
                        # half projection between the two heads of the second
                        # token-half (its collective completed ~15us ago);
                        # earlier batches push this slab to the next section
                        # start, where readiness is unconditional
                        if jh == 1 and h == 0 and b == B - 1:
                            emit_proj(b - 1, 1)

                    # both heads of this token-half are done (chunks were
                    # scattered group-by-group above): kick its AllToAll; it
                    # overlaps the remaining attention
                    nc.gpsimd.collective_compute(
                        "AllToAll",
                        mybir.AluOpType.bypass,
                        replica_groups=[list(range(NCORES))],
                        ins=[a2a_in[b][jh][:]],
                        outs=[a2a_out[b][jh][:]],
                    )
                    # last batch: its first-half projection fills the start
                    # of the exposed final-collective wait (its own
                    # collective completed during this second half)
                    if jh == 1 and b == B - 1:
                        emit_proj(b, 0)

                # schedule: full QKV, then the previous batch's first-half
                # projection (its collective is long done), then attention.
                # (Interleaving the second half's QKV into the first
                # attention half was tried — with per-half tiles so no false
                # dependency — and regressed ~8us: it delays the second
                # attention half, and the cross-phase ACT/PE overlap it
                # targets already happens via natural pipelining.)
                for tch in range(T // 512):
                    emit_qkv_tch(tch)
                if b > 1:
                    emit_proj(b - 2, 1)
                if b > 0:
                    emit_proj(b - 1, 0)
                emit_attn_half(0)
                emit_attn_half(1)
              emit_proj(B - 1, 1)

    if split:
        split_waits(nc)
    return nc


def _get_nc_fused():
    if "fused" not in _CACHE:
        _CACHE["fused"] = _build_fused()
    return _CACHE["fused"]


def make_in_maps(x, wq, wk, wv, w_proj):
    xT = np.ascontiguousarray(
        np.asarray(x, np.float32).reshape(BT, C).T
    ).astype(BF)
    wpb = np.ascontiguousarray(np.asarray(w_proj, np.float32)).astype(BF)
    scale = np.float32(C) ** -0.5
    in_maps = []
    for c in range(NCORES):
        h0, h1 = HPC * c, HPC * c + 1
        in_maps.append(
            {
                "xT": xT,
                "wq2": np.ascontiguousarray(
                    np.concatenate([wq[h0] * scale, wq[h1] * scale], axis=1)
                ).astype(BF),
                "wk2": np.ascontiguousarray(
                    np.concatenate([wk[h0], wk[h1]], axis=1)
                ).astype(BF),
                "wv2": np.ascontiguousarray(
                    np.concatenate([wv[h0], wv[h1]], axis=1)
                ).astype(BF),
                "wp": wpb,
            }
        )
    return in_maps


def assemble_output(results, b_proj):
    y = np.empty((BT, C), dtype=np.float32)
    for c in range(NCORES):
        yTc = results[c]["yT"]
        for b in range(B):
            for jh in range(2):
                g0 = T * b + 1024 * jh + 128 * c
                s0 = 256 * b + 128 * jh
                y[g0 : g0 + 128, :] = yTc[:, s0 : s0 + 128].T
    y = y.reshape(B, T, C)
    y += np.asarray(b_proj, np.float32)
    return y


def kernel(x, wq, wk, wv, w_proj, b_proj):
    wq = np.asarray(wq, np.float32)
    wk = np.asarray(wk, np.float32)
    wv = np.asarray(wv, np.float32)

    res = run_bass_kernel_spmd(
        _get_nc_fused(),
        make_in_maps(x, wq, wk, wv, w_proj),
        core_ids=list(range(NCORES)),
    )
    return assemble_output(res.results, b_proj)
